# revision 2
# baseline (speedup 1.0000x reference)
"""Multi-head self-attention 2d (B=2, C=256, H=W=64, 8 heads x 32 dim) on 8 TRN2 cores.

Sharding: batch (2-way) x query-rows-of-N=H*W (4-way) => 8 cores, no collectives.
v5: HAM-friendly quad boundaries --
  - the four (query-half, head-group) quads run as one continuous 128-iteration
    score/exp/AV pipeline; the next quad's score matmuls are emitted during the
    current quad's last iteration,
  - at each quad end the two PSUM accumulators are evacuated to SBUF with two
    quick copies (Op banks free in ~1.5us instead of ~4us), the reciprocal runs
    from SBUF with a -32-partition shift so every normalize multiply is
    partition-aligned, and the multiplies run on the otherwise-idle GpSimd
    engine; this kills the PE idle window at boundaries that was tripping the
    HAM clock throttle (PE at half clock for ~40us in v4),
  - normalized outputs stay in the natural PSUM row layout (denominator rows
    interleaved); the projection weights are zero-padded host-side at those
    rows so the output projection contracts over 512 rows with 4 matmuls,
  - xb is rotated per-core on the host so the query block is always columns
    0-1023 (key order is irrelevant to attention), keeping the program SPMD,
  - all weights ride in one [128, 2560] DMA; xb's first 1024 columns (feeding
    Q-proj and K chunks 0-1) transfer first,
  - K/V projections interleaved into the first quad, stealing score PSUM slots,
  - V stored as [V_h | ones32] so the AV matmul emits denominators pre-broadcast
    into 32 PSUM rows.
"""

import os
import sys

import numpy as np

for _p in ("/opt/trn_rl_repo", "/root/.axon_site/_ro/trn_rl_repo"):
    if os.path.isdir(_p) and _p not in sys.path:
        sys.path.insert(0, _p)

import ml_dtypes
import concourse.bacc as bacc
import concourse.bass as bass
import concourse.tile as tile
from concourse import mybir
from concourse.bass_utils import run_bass_kernel_spmd

BF16 = mybir.dt.bfloat16
F32 = mybir.dt.float32
I16 = mybir.dt.int16
NPBF16 = ml_dtypes.bfloat16

NH, D = 8, 32          # heads, head dim
C = 256                # channels
N = 4096               # H*W positions
Q = 1024               # query shard per core
SCALE = 1.0 / np.sqrt(D)

# Schraudolph bf16 exp on the vector engine: bf16_bits(exp(y)) ~= round(y*128/ln2
# + (127*128 - c)). Fold the attention scale into the multiplier. c tuned for
# min max-rel-error under round-to-nearest (~3.3%).
SCH_A = float(SCALE * 128.0 / np.log(2.0))
SCH_B = float(127.0 * 128.0 - 5.5)
# Fraction (num/32) of exp tiles on ScalarE: [quad0, steady]; quad 0 is
# ACT-heavy because DVE also drains the interleaved K/V projection copies.
ACT_UNITS = [21, 17]
TAIL_M = 30            # m >= TAIL_M goes all-ACT (DVE runs the normalize there)
TAIL_UNITS = 32
HEAD_M = 4             # first HEAD_M m-iters of quads 1-3 lean ACT (+HEAD_BUMP)
HEAD_BUMP = 10


def _build_program():
    nc = bacc.Bacc("TRN2", target_bir_lowering=False, debug=False)

    xb = nc.dram_tensor("xb", [C, N], BF16, kind="ExternalInput")
    xq = nc.dram_tensor("xq", [C, Q], F32, kind="ExternalInput")
    wall = nc.dram_tensor("wall", [128, 2560], BF16, kind="ExternalInput")
    gam = nc.dram_tensor("gam", [128, 1], F32, kind="ExternalInput")
    out = nc.dram_tensor("out", [C, Q], F32, kind="ExternalOutput")

    with tile.TileContext(nc) as tc:
        _emit(tc, xb, xq, wall, gam, out)
    nc.compile()
    return nc


def _emit(tc, xb, xq, wall, gam, out):
    from contextlib import ExitStack

    nc = tc.nc
    Exp = mybir.ActivationFunctionType.Exp

    with ExitStack() as ctx:
        per = ctx.enter_context(tc.tile_pool(name="persist", bufs=1))

        def ptile(name, shape, dtype):
            return per.tile(shape, dtype, name=name, tag=name)

        XB = [ptile(f"XB{i}", [128, N], BF16) for i in range(2)]
        XQ = [ptile(f"XQ{i}", [128, Q], F32) for i in range(2)]
        W = ptile("W", [128, 2560], BF16)   # wq01 wk01 wv01 pj[hg=0,j=0..1] pj[hg=1,...]
        WQ = [W[:, 256 * c:256 * (c + 1)] for c in range(2)]
        WK = [W[:, 512 + 256 * c:512 + 256 * (c + 1)] for c in range(2)]
        WV = [W[:, 1024 + 256 * c:1024 + 256 * (c + 1)] for c in range(2)]
        PJP = [W[:, 1536 + 256 * g:1536 + 256 * (g + 1)] for g in range(4)]  # (2hg+j)
        G = ptile("G", [128, 1], F32)
        Ksb = ptile("Ksb", [128, 2 * N], BF16)      # [dim-in-group, hg*4096 + key]
        Qsb = ptile("Qsb", [128, 2 * Q], BF16)      # [dim-in-group, hg*1024 + q]
        Vsb = ptile("Vsb", [128, 32 * 512], BF16)   # per m-chunk: 8 x [V(32)|ones(32)]
        # normalized attention outputs, natural PSUM row layout per (hg, j):
        # rows 0-31 head 2j, 32-63 junk, 64-95 head 2j+1, 96-127 junk
        OsbR = [ptile(f"OsbR{i}", [128, 2 * Q], BF16) for i in range(2)]

        # DMAs spread across engine DGE queues so the transfers run in
        # parallel (a single queue serializes at ~1.3us+transfer per DMA):
        # sync + scalar carry the two xb halves, vector carries the weights
        # (wq/wk first -- they gate Q/K-proj), gpsimd (idle, behind the
        # memsets) carries the late-needed xq/gam.
        nc.scalar.dma_start(W[:, 0:1024], wall[:, 0:1024])
        nc.sync.dma_start(XB[0][:, 0:1024], xb[0:128, 0:1024])
        nc.gpsimd.dma_start(XB[1][:, 0:1024], xb[128:256, 0:1024])
        nc.scalar.dma_start(W[:, 1024:2560], wall[:, 1024:2560])
        nc.sync.dma_start(XB[0][:, 1024:4096], xb[0:128, 1024:4096])
        nc.sync.dma_start(XB[1][:, 1024:4096], xb[128:256, 1024:4096])

        # one-time SBUF init on the idle GpSimd engine (behind its xb DMA):
        # ones blocks of Vsb and the junk rows of OsbR (so 0-padded proj rows
        # never hit NaN*0); then the late-needed xq/gam DMAs.
        v4 = Vsb.rearrange("p (mh w) -> p mh w", w=64)
        for m in range(32):
            nc.gpsimd.memset(v4[:, 8 * m:8 * (m + 1), 32:64], 1.0)
        for i in range(2):
            nc.gpsimd.memset(OsbR[i][32:64, :], 0.0)
            nc.gpsimd.memset(OsbR[i][96:128, :], 0.0)
        for i in range(2):
            r = slice(128 * i, 128 * (i + 1))
            nc.gpsimd.dma_start(XQ[i][:], xq[r, :])
        nc.gpsimd.dma_start(G[:], gam[:, :])

        exp_idx = [0]

        with ExitStack() as actx:
            sp = actx.enter_context(tc.tile_pool(name="sp", bufs=3, space="PSUM"))
            opl = actx.enter_context(tc.tile_pool(name="opl", bufs=1, space="PSUM"))
            pb = actx.enter_context(tc.tile_pool(name="pb", bufs=6))
            osb = actx.enter_context(tc.tile_pool(name="osb", bufs=2))
            rb = actx.enter_context(tc.tile_pool(name="rb", bufs=2))
            ob = actx.enter_context(tc.tile_pool(name="ob", bufs=2))

            def slot(name):
                return sp.tile([128, 1024], F32, name=name, tag="st2")

            def emit_qproj(p):
                qp = slot(f"qp{p}")
                for t2 in range(2):
                    ts_ = slice(512 * t2, 512 * (t2 + 1))
                    for c in range(2):
                        nc.tensor.matmul(qp[:, ts_], lhsT=WQ[c][:, 128 * p:128 * (p + 1)],
                                         rhs=XB[c][:, ts_], start=(c == 0), stop=(c == 1))
                nc.vector.tensor_copy(Qsb[:, 1024 * p:1024 * (p + 1)], qp[:])

            def emit_kproj(t):
                kp = slot(f"kp{t}")
                xs = slice(512 * t, 512 * (t + 1))
                for p in range(2):
                    ps_ = slice(512 * p, 512 * (p + 1))
                    for c in range(2):
                        nc.tensor.matmul(kp[:, ps_], lhsT=WK[c][:, 128 * p:128 * (p + 1)],
                                         rhs=XB[c][:, xs], start=(c == 0), stop=(c == 1))
                k3 = Ksb.rearrange("p (h w) -> p h w", w=N)
                nc.vector.tensor_copy(k3[:, :, 512 * t:512 * (t + 1)],
                                      kp[:].rearrange("p (h w) -> p h w", w=512))

            def emit_vproj4(mq):
                # chunks 4mq..4mq+3 packed into one slot at columns 256k
                vp = slot(f"vp{mq}")
                for k in range(4):
                    m = 4 * mq + k
                    ms = slice(128 * m, 128 * (m + 1))
                    vs = slice(256 * k, 256 * (k + 1))
                    nc.tensor.matmul(vp[:, vs], lhsT=XB[0][:, ms], rhs=WV[0][:],
                                     start=True, stop=False)
                    nc.tensor.matmul(vp[:, vs], lhsT=XB[1][:, ms], rhs=WV[1][:],
                                     start=False, stop=True)
                v3 = Vsb.rearrange("p (mh w) -> p mh w", w=64)
                nc.vector.tensor_copy(
                    v3[:, 32 * mq:32 * (mq + 1), 0:32],
                    vp[:].rearrange("p (kh d) -> p kh d", d=32))

            # flattened quad sequence: (qh, hg, units)
            quads = [(0, 0, ACT_UNITS[0]), (0, 1, ACT_UNITS[1]),
                     (1, 0, ACT_UNITS[1]), (1, 1, ACT_UNITS[1])]
            pts_by = {}
            Op_by = {}

            def emit_s(qi, m):
                qh, hg, units = quads[qi]
                if m >= TAIL_M:
                    u = TAIL_UNITS
                elif m < HEAD_M and qi > 0:
                    u = min(32, units + HEAD_BUMP)
                else:
                    u = units
                sts = [slot("st2s") for _ in range(2)]
                for g in range(2):
                    for j in range(2):
                        a = 2 * g + j
                        hh = slice(32 * a, 32 * (a + 1))
                        nc.tensor.matmul(
                            sts[g][:, 512 * j:512 * (j + 1)],
                            lhsT=Ksb[hh, N * hg + 128 * m:N * hg + 128 * (m + 1)],
                            rhs=Qsb[hh, Q * hg + 512 * qh:Q * hg + 512 * (qh + 1)],
                            start=True, stop=True,
                            tile_position=(32 * a, 0))
                pts = []
                for g in range(2):
                    pt2 = pb.tile([128, 1024], BF16, name="pt2", tag="pt2")
                    pts.append(pt2)
                    k = exp_idx[0]
                    exp_idx[0] += 1
                    if (k * u) % 32 < u:
                        nc.scalar.activation(pt2[:], sts[g][:], Exp, scale=SCALE)
                    else:
                        nc.vector.tensor_scalar(
                            pt2.bitcast(I16)[:], sts[g][:], SCH_A, SCH_B,
                            mybir.AluOpType.mult, mybir.AluOpType.add)
                pts_by[(qi, m)] = pts

            def emit_av(qi, m):
                qh, hg, _ = quads[qi]
                if m == 0:
                    Op_by[qi] = [opl.tile([128, 512], F32, name=f"Op{j}", tag=f"Op{j}")
                                 for j in range(2)]
                Op = Op_by[qi]
                pts = pts_by.pop((qi, m))
                first, last = m == 0, m == 31
                for j in range(2):
                    for b in range(2):
                        a = 2 * j + b
                        H = 4 * hg + a
                        nc.tensor.matmul(
                            Op[j][64 * b:64 * (b + 1), :],
                            lhsT=Vsb[:, 512 * m + 64 * H:512 * m + 64 * (H + 1)],
                            rhs=pts[j][:, 512 * b:512 * (b + 1)],
                            start=first, stop=last,
                            tile_position=(0, 64 * b), skip_group_check=True)

            def emit_norm(qi):
                # rows 0-31: O head 2j; 32-63: denom head 2j (pre-broadcast by
                # the ones32 block of V); 64-95: O head 2j+1; 96-127: denom.
                qh, hg, _ = quads[qi]
                qs = slice(512 * qh, 512 * (qh + 1))
                Op = Op_by.pop(qi)
                if False:
                    # evacuate PSUM fast (Op banks free after 2 copies); then,
                    # off the Op critical path: shift the denominator rows down
                    # 32 with plain copies (custom DVE ops break on partition
                    # shifts; SBUF-SBUF tensor_tensor needs equal bases),
                    # aligned reciprocal, and fully-aligned GpSimd multiplies.
                    OS = osb.tile([128, 1024], F32, name="OS", tag="OS")
                    for j in range(2):
                        nc.vector.tensor_copy(OS[:, 512 * j:512 * (j + 1)], Op[j][:])
                    dn = rb.tile([128, 1024], F32, name="dn", tag="dn")
                    rj = rb.tile([128, 1024], F32, name="rj", tag="rj")
                    for b in range(2):
                        nc.vector.tensor_copy(dn[64 * b:64 * b + 32, :],
                                              OS[64 * b + 32:64 * b + 64, :])
                    # single base-0 op: reciprocal_approx_fast misreads at
                    # partition base 64 (rows 32-63 are junk-in, junk-out)
                    nc.vector.reciprocal_approx_fast(out=rj[0:96, :], in_=dn[0:96, :])
                    for j in range(2):
                        for b in range(2):
                            rr = slice(64 * b, 64 * b + 32)
                            nc.gpsimd.tensor_tensor(
                                OsbR[hg][rr, Q * j + 512 * qh:Q * j + 512 * (qh + 1)],
                                OS[rr, 512 * j:512 * j + 512],
                                rj[rr, 512 * j:512 * j + 512],
                                mybir.AluOpType.mult)
                else:
                    # tail quad: direct from PSUM on DVE (shortest latency)
                    for j in range(2):
                        rj = rb.tile([128, 1024], F32, name="rj", tag="rj")
                        nc.vector.reciprocal_approx_fast(out=rj[:, :512], in_=Op[j][:])
                        for b in range(2):
                            nc.vector.tensor_tensor(
                                OsbR[hg][64 * b:64 * b + 32,
                                         Q * j + 512 * qh:Q * j + 512 * (qh + 1)],
                                Op[j][64 * b:64 * b + 32, :],
                                rj[64 * b + 32:64 * b + 64, :512],
                                mybir.AluOpType.mult)

            def emit_outproj(qh):
                qs = slice(512 * qh, 512 * (qh + 1))
                for ct in range(2):
                    cs = slice(128 * ct, 128 * (ct + 1))
                    pp2 = slot(f"op{ct}")
                    for g in range(4):          # g = 2*hg + j
                        hg, j = divmod(g, 2)
                        nc.tensor.matmul(
                            pp2[:, :512], lhsT=PJP[g][:, cs],
                            rhs=OsbR[hg][:, Q * j + 512 * qh:Q * j + 512 * (qh + 1)],
                            start=(g == 0), stop=(g == 3))
                    obt = ob.tile([128, 512], F32, name="obt", tag="obt")
                    nc.vector.scalar_tensor_tensor(
                        obt[:], pp2[:, :512], G[:], XQ[ct][:, qs],
                        mybir.AluOpType.mult, mybir.AluOpType.add)
                    nc.sync.dma_start(out[cs, qs], obt[:])

            # projections needed up front: Q (both groups), K chunks 0-1, first
            # V quad-chunk; the rest interleaves into quad 0.
            emit_qproj(0)
            emit_qproj(1)
            emit_kproj(0)
            emit_s(0, 0)
            emit_kproj(1)
            emit_vproj4(0)
            for it in range(128):
                qi, m = divmod(it, 32)
                if qi == 0:
                    if m % 4 == 0 and m // 4 + 2 < 8:
                        emit_kproj(m // 4 + 2)
                    if m % 4 == 1 and m + 3 < 32:
                        emit_vproj4((m + 3) // 4)
                if it + 1 < 128:
                    emit_s(*divmod(it + 1, 32))
                emit_av(qi, m)
                if m == 31:
                    emit_norm(qi)
                    if qi == 1:
                        emit_outproj(0)
                    elif qi == 3:
                        emit_outproj(1)


_NC = None


def _get_program():
    global _NC
    if _NC is None:
        _NC = _build_program()
    return _NC


def kernel(x, qkv_w, proj_w, gamma, _trace=False):
    """Full inputs in, full output out. Shards across 8 NeuronCores internally."""
    nc = _get_program()
    B = x.shape[0]
    xf = np.ascontiguousarray(x.reshape(B, C, N).astype(np.float32))
    xf_bf = xf.astype(NPBF16)

    wqT = qkv_w[0:256].T.astype(NPBF16)
    wkT = qkv_w[256:512].T.astype(NPBF16)
    wvT = qkv_w[512:768].T.astype(NPBF16)
    pjT = proj_w.T.astype(NPBF16)
    # zero-padded proj tiles in the natural PSUM row layout of OsbR: for
    # g = 2*hg + j: rows 0-31 = head (4hg+2j) dims, 64-95 = head (4hg+2j+1)
    pjp = np.zeros((4, 128, 256), dtype=NPBF16)
    for g in range(4):
        hg, j = divmod(g, 2)
        h0 = 4 * hg + 2 * j
        pjp[g][0:32] = pjT[32 * h0:32 * (h0 + 1)]
        pjp[g][64:96] = pjT[32 * (h0 + 1):32 * (h0 + 2)]
    wall = np.ascontiguousarray(np.concatenate(
        [wqT[0:128], wqT[128:256], wkT[0:128], wkT[128:256],
         wvT[0:128], wvT[128:256], pjp[0], pjp[1], pjp[2], pjp[3]], axis=1))
    gam = np.full((128, 1), np.float32(gamma.reshape(-1)[0]), dtype=np.float32)

    in_maps = []
    for core in range(8):
        b, qi = divmod(core, 4)
        qs = slice(Q * qi, Q * (qi + 1))
        # rotate keys so this core's query block sits at columns 0-1023; key
        # order is irrelevant to attention (softmax + sum over keys).
        xrot = np.roll(xf_bf[b], -Q * qi, axis=1) if qi else xf_bf[b]
        in_maps.append({
            "xb": np.ascontiguousarray(xrot),
            "xq": np.ascontiguousarray(xf[b][:, qs]),
            "wall": wall,
            "gam": gam,
        })

    res = run_bass_kernel_spmd(nc, in_maps, core_ids=list(range(8)), trace=_trace)

    outf = np.empty((B, C, N), dtype=np.float32)
    for core in range(8):
        b, qi = divmod(core, 4)
        outf[b][:, Q * qi:Q * (qi + 1)] = res.results[core]["out"]
    result = outf.reshape(x.shape)
    if _trace:
        return result, res
    return result


# revision 3
# speedup vs baseline: 1.0318x; 1.0318x over previous
"""Multi-head self-attention 2d (B=2, C=256, H=W=64, 8 heads x 32 dim) on 8 TRN2 cores.

Sharding: batch (2-way) x query-rows-of-N=H*W (4-way) => 8 cores, no collectives.
v5: HAM-friendly quad boundaries --
  - the four (query-half, head-group) quads run as one continuous 128-iteration
    score/exp/AV pipeline; the next quad's score matmuls are emitted during the
    current quad's last iteration,
  - at each quad end the two PSUM accumulators are evacuated to SBUF with two
    quick copies (Op banks free in ~1.5us instead of ~4us), the reciprocal runs
    from SBUF with a -32-partition shift so every normalize multiply is
    partition-aligned, and the multiplies run on the otherwise-idle GpSimd
    engine; this kills the PE idle window at boundaries that was tripping the
    HAM clock throttle (PE at half clock for ~40us in v4),
  - normalized outputs stay in the natural PSUM row layout (denominator rows
    interleaved); the projection weights are zero-padded host-side at those
    rows so the output projection contracts over 512 rows with 4 matmuls,
  - xb is rotated per-core on the host so the query block is always columns
    0-1023 (key order is irrelevant to attention), keeping the program SPMD,
  - all weights ride in one [128, 2560] DMA; xb's first 1024 columns (feeding
    Q-proj and K chunks 0-1) transfer first,
  - K/V projections interleaved into the first quad, stealing score PSUM slots,
  - V stored as [V_h | ones32] so the AV matmul emits denominators pre-broadcast
    into 32 PSUM rows.
"""

import os
import sys

import numpy as np

for _p in ("/opt/trn_rl_repo", "/root/.axon_site/_ro/trn_rl_repo"):
    if os.path.isdir(_p) and _p not in sys.path:
        sys.path.insert(0, _p)

import ml_dtypes
import concourse.bacc as bacc
import concourse.bass as bass
import concourse.tile as tile
from concourse import mybir
from concourse.bass_utils import run_bass_kernel_spmd

BF16 = mybir.dt.bfloat16
F32 = mybir.dt.float32
I16 = mybir.dt.int16
NPBF16 = ml_dtypes.bfloat16

NH, D = 8, 32          # heads, head dim
C = 256                # channels
N = 4096               # H*W positions
Q = 1024               # query shard per core
SCALE = 1.0 / np.sqrt(D)

# Schraudolph bf16 exp on the vector engine: bf16_bits(exp(y)) ~= round(y*128/ln2
# + (127*128 - c)). Fold the attention scale into the multiplier. c tuned for
# min max-rel-error under round-to-nearest (~3.3%).
SCH_A = float(SCALE * 128.0 / np.log(2.0))
SCH_B = float(127.0 * 128.0 - 5.5)
# Fraction (num/32) of exp tiles on ScalarE: [quad0, steady]; quad 0 is
# ACT-heavy because DVE also drains the interleaved K/V projection copies.
ACT_UNITS = [21, 17]
TAIL_M = 31            # m >= TAIL_M goes all-ACT (DVE runs the normalize there)
TAIL_UNITS = 32
HEAD_M = 4             # first HEAD_M m-iters of quads 1-3 lean ACT (+HEAD_BUMP)
HEAD_BUMP = 10


def _build_program():
    nc = bacc.Bacc("TRN2", target_bir_lowering=False, debug=False)

    xb = nc.dram_tensor("xb", [C, N], BF16, kind="ExternalInput")
    xq = nc.dram_tensor("xq", [C, Q], F32, kind="ExternalInput")
    wall = nc.dram_tensor("wall", [128, 2560], BF16, kind="ExternalInput")
    gam = nc.dram_tensor("gam", [128, 1], F32, kind="ExternalInput")
    out = nc.dram_tensor("out", [C, Q], F32, kind="ExternalOutput")

    with tile.TileContext(nc) as tc:
        _emit(tc, xb, xq, wall, gam, out)
    nc.compile()
    return nc


def _emit(tc, xb, xq, wall, gam, out):
    from contextlib import ExitStack

    nc = tc.nc
    Exp = mybir.ActivationFunctionType.Exp

    with ExitStack() as ctx:
        per = ctx.enter_context(tc.tile_pool(name="persist", bufs=1))

        def ptile(name, shape, dtype):
            return per.tile(shape, dtype, name=name, tag=name)

        XB = [ptile(f"XB{i}", [128, N], BF16) for i in range(2)]
        XQ = [ptile(f"XQ{i}", [128, Q], F32) for i in range(2)]
        W = ptile("W", [128, 2560], BF16)   # wq01 wk01 wv01 pj[hg=0,j=0..1] pj[hg=1,...]
        WQ = [W[:, 256 * c:256 * (c + 1)] for c in range(2)]
        WK = [W[:, 512 + 256 * c:512 + 256 * (c + 1)] for c in range(2)]
        WV = [W[:, 1024 + 256 * c:1024 + 256 * (c + 1)] for c in range(2)]
        PJP = [W[:, 1536 + 256 * g:1536 + 256 * (g + 1)] for g in range(4)]  # (2hg+j)
        G = ptile("G", [128, 1], F32)
        Ksb = ptile("Ksb", [128, 2 * N], BF16)      # [dim-in-group, hg*4096 + key]
        Qsb = ptile("Qsb", [128, 2 * Q], BF16)      # [dim-in-group, hg*1024 + q]
        Vsb = ptile("Vsb", [128, 32 * 512], BF16)   # per m-chunk: 8 x [V(32)|ones(32)]
        # normalized attention outputs, natural PSUM row layout per (hg, j):
        # rows 0-31 head 2j, 32-63 junk, 64-95 head 2j+1, 96-127 junk
        OsbR = [ptile(f"OsbR{i}", [128, 2 * Q], BF16) for i in range(2)]

        # DMAs spread across engine DGE queues so the transfers run in
        # parallel (a single queue serializes at ~1.3us+transfer per DMA):
        # sync + scalar carry the two xb halves, vector carries the weights
        # (wq/wk first -- they gate Q/K-proj), gpsimd (idle, behind the
        # memsets) carries the late-needed xq/gam.
        nc.scalar.dma_start(W[:, 0:1024], wall[:, 0:1024])
        nc.sync.dma_start(XB[0][:, 0:1024], xb[0:128, 0:1024])
        nc.gpsimd.dma_start(XB[1][:, 0:1024], xb[128:256, 0:1024])
        nc.scalar.dma_start(W[:, 1024:2560], wall[:, 1024:2560])
        nc.sync.dma_start(XB[0][:, 1024:4096], xb[0:128, 1024:4096])
        nc.sync.dma_start(XB[1][:, 1024:4096], xb[128:256, 1024:4096])

        # one-time SBUF init on the idle GpSimd engine (behind its xb DMA):
        # ones blocks of Vsb and the junk rows of OsbR (so 0-padded proj rows
        # never hit NaN*0); then the late-needed xq/gam DMAs.
        v4 = Vsb.rearrange("p (mh w) -> p mh w", w=64)
        for m in range(32):
            nc.gpsimd.memset(v4[:, 8 * m:8 * (m + 1), 32:64], 1.0)
        for i in range(2):
            nc.gpsimd.memset(OsbR[i][32:64, :], 0.0)
            nc.gpsimd.memset(OsbR[i][96:128, :], 0.0)
        for i in range(2):
            r = slice(128 * i, 128 * (i + 1))
            nc.gpsimd.dma_start(XQ[i][:], xq[r, :])
        nc.gpsimd.dma_start(G[:], gam[:, :])

        exp_idx = [0]

        with ExitStack() as actx:
            sp = actx.enter_context(tc.tile_pool(name="sp", bufs=3, space="PSUM"))
            opl = actx.enter_context(tc.tile_pool(name="opl", bufs=1, space="PSUM"))
            pb = actx.enter_context(tc.tile_pool(name="pb", bufs=8))
            osb = actx.enter_context(tc.tile_pool(name="osb", bufs=2))
            rb = actx.enter_context(tc.tile_pool(name="rb", bufs=2))
            ob = actx.enter_context(tc.tile_pool(name="ob", bufs=2))

            def slot(name):
                return sp.tile([128, 1024], F32, name=name, tag="st2")

            def emit_qproj(p):
                qp = slot(f"qp{p}")
                for t2 in range(2):
                    ts_ = slice(512 * t2, 512 * (t2 + 1))
                    for c in range(2):
                        nc.tensor.matmul(qp[:, ts_], lhsT=WQ[c][:, 128 * p:128 * (p + 1)],
                                         rhs=XB[c][:, ts_], start=(c == 0), stop=(c == 1))
                nc.vector.tensor_copy(Qsb[:, 1024 * p:1024 * (p + 1)], qp[:])

            def emit_kproj(t):
                kp = slot(f"kp{t}")
                xs = slice(512 * t, 512 * (t + 1))
                for p in range(2):
                    ps_ = slice(512 * p, 512 * (p + 1))
                    for c in range(2):
                        nc.tensor.matmul(kp[:, ps_], lhsT=WK[c][:, 128 * p:128 * (p + 1)],
                                         rhs=XB[c][:, xs], start=(c == 0), stop=(c == 1))
                k3 = Ksb.rearrange("p (h w) -> p h w", w=N)
                nc.vector.tensor_copy(k3[:, :, 512 * t:512 * (t + 1)],
                                      kp[:].rearrange("p (h w) -> p h w", w=512))

            def emit_vproj4(mq):
                # chunks 4mq..4mq+3 packed into one slot at columns 256k
                vp = slot(f"vp{mq}")
                for k in range(4):
                    m = 4 * mq + k
                    ms = slice(128 * m, 128 * (m + 1))
                    vs = slice(256 * k, 256 * (k + 1))
                    nc.tensor.matmul(vp[:, vs], lhsT=XB[0][:, ms], rhs=WV[0][:],
                                     start=True, stop=False)
                    nc.tensor.matmul(vp[:, vs], lhsT=XB[1][:, ms], rhs=WV[1][:],
                                     start=False, stop=True)
                v3 = Vsb.rearrange("p (mh w) -> p mh w", w=64)
                nc.vector.tensor_copy(
                    v3[:, 32 * mq:32 * (mq + 1), 0:32],
                    vp[:].rearrange("p (kh d) -> p kh d", d=32))

            # flattened quad sequence: (qh, hg, units)
            quads = [(0, 0, ACT_UNITS[0]), (0, 1, ACT_UNITS[1]),
                     (1, 0, ACT_UNITS[1]), (1, 1, ACT_UNITS[1])]
            pts_by = {}
            Op_by = {}

            def emit_s(qi, m):
                qh, hg, units = quads[qi]
                if m >= TAIL_M:
                    u = TAIL_UNITS
                elif m < HEAD_M and qi > 0:
                    u = min(32, units + HEAD_BUMP)
                else:
                    u = units
                sts = [slot("st2s") for _ in range(2)]
                for g in range(2):
                    for j in range(2):
                        a = 2 * g + j
                        hh = slice(32 * a, 32 * (a + 1))
                        nc.tensor.matmul(
                            sts[g][:, 512 * j:512 * (j + 1)],
                            lhsT=Ksb[hh, N * hg + 128 * m:N * hg + 128 * (m + 1)],
                            rhs=Qsb[hh, Q * hg + 512 * qh:Q * hg + 512 * (qh + 1)],
                            start=True, stop=True,
                            tile_position=(32 * a, 0))
                pts = []
                for g in range(2):
                    pt2 = pb.tile([128, 1024], BF16, name="pt2", tag="pt2")
                    pts.append(pt2)
                    k = exp_idx[0]
                    exp_idx[0] += 1
                    if (k * u) % 32 < u:
                        nc.scalar.activation(pt2[:], sts[g][:], Exp, scale=SCALE)
                    else:
                        nc.vector.tensor_scalar(
                            pt2.bitcast(I16)[:], sts[g][:], SCH_A, SCH_B,
                            mybir.AluOpType.mult, mybir.AluOpType.add)
                pts_by[(qi, m)] = pts

            def emit_av(qi, m):
                qh, hg, _ = quads[qi]
                if m == 0:
                    Op_by[qi] = [opl.tile([128, 512], F32, name=f"Op{j}", tag=f"Op{j}")
                                 for j in range(2)]
                Op = Op_by[qi]
                pts = pts_by.pop((qi, m))
                first, last = m == 0, m == 31
                for j in range(2):
                    for b in range(2):
                        a = 2 * j + b
                        H = 4 * hg + a
                        nc.tensor.matmul(
                            Op[j][64 * b:64 * (b + 1), :],
                            lhsT=Vsb[:, 512 * m + 64 * H:512 * m + 64 * (H + 1)],
                            rhs=pts[j][:, 512 * b:512 * (b + 1)],
                            start=first, stop=last,
                            tile_position=(0, 64 * b), skip_group_check=True)

            def emit_norm(qi):
                # rows 0-31: O head 2j; 32-63: denom head 2j (pre-broadcast by
                # the ones32 block of V); 64-95: O head 2j+1; 96-127: denom.
                qh, hg, _ = quads[qi]
                qs = slice(512 * qh, 512 * (qh + 1))
                Op = Op_by.pop(qi)
                if False:
                    # evacuate PSUM fast (Op banks free after 2 copies); then,
                    # off the Op critical path: shift the denominator rows down
                    # 32 with plain copies (custom DVE ops break on partition
                    # shifts; SBUF-SBUF tensor_tensor needs equal bases),
                    # aligned reciprocal, and fully-aligned GpSimd multiplies.
                    OS = osb.tile([128, 1024], F32, name="OS", tag="OS")
                    for j in range(2):
                        nc.vector.tensor_copy(OS[:, 512 * j:512 * (j + 1)], Op[j][:])
                    dn = rb.tile([128, 1024], F32, name="dn", tag="dn")
                    rj = rb.tile([128, 1024], F32, name="rj", tag="rj")
                    for b in range(2):
                        nc.vector.tensor_copy(dn[64 * b:64 * b + 32, :],
                                              OS[64 * b + 32:64 * b + 64, :])
                    # single base-0 op: reciprocal_approx_fast misreads at
                    # partition base 64 (rows 32-63 are junk-in, junk-out)
                    nc.vector.reciprocal_approx_fast(out=rj[0:96, :], in_=dn[0:96, :])
                    for j in range(2):
                        for b in range(2):
                            rr = slice(64 * b, 64 * b + 32)
                            nc.gpsimd.tensor_tensor(
                                OsbR[hg][rr, Q * j + 512 * qh:Q * j + 512 * (qh + 1)],
                                OS[rr, 512 * j:512 * j + 512],
                                rj[rr, 512 * j:512 * j + 512],
                                mybir.AluOpType.mult)
                else:
                    # tail quad: direct from PSUM on DVE (shortest latency)
                    for j in range(2):
                        rj = rb.tile([128, 1024], F32, name="rj", tag="rj")
                        nc.vector.reciprocal_approx_fast(out=rj[:, :512], in_=Op[j][:])
                        for b in range(2):
                            nc.vector.tensor_tensor(
                                OsbR[hg][64 * b:64 * b + 32,
                                         Q * j + 512 * qh:Q * j + 512 * (qh + 1)],
                                Op[j][64 * b:64 * b + 32, :],
                                rj[64 * b + 32:64 * b + 64, :512],
                                mybir.AluOpType.mult)

            def emit_outproj(qh):
                qs = slice(512 * qh, 512 * (qh + 1))
                for ct in range(2):
                    cs = slice(128 * ct, 128 * (ct + 1))
                    pp2 = slot(f"op{ct}")
                    for g in range(4):          # g = 2*hg + j
                        hg, j = divmod(g, 2)
                        nc.tensor.matmul(
                            pp2[:, :512], lhsT=PJP[g][:, cs],
                            rhs=OsbR[hg][:, Q * j + 512 * qh:Q * j + 512 * (qh + 1)],
                            start=(g == 0), stop=(g == 3))
                    obt = ob.tile([128, 512], F32, name="obt", tag="obt")
                    nc.vector.scalar_tensor_tensor(
                        obt[:], pp2[:, :512], G[:], XQ[ct][:, qs],
                        mybir.AluOpType.mult, mybir.AluOpType.add)
                    nc.sync.dma_start(out[cs, qs], obt[:])

            # projections needed up front: Q (both groups), K chunks 0-1, first
            # V quad-chunk; the rest interleaves into quad 0.
            emit_qproj(0)
            emit_qproj(1)
            emit_kproj(0)
            emit_s(0, 0)
            emit_kproj(1)
            emit_vproj4(0)
            for it in range(128):
                qi, m = divmod(it, 32)
                if qi == 0:
                    if m % 4 == 2 and m // 4 + 2 < 8:
                        emit_kproj(m // 4 + 2)
                    if m % 4 == 1 and m + 3 < 32:
                        emit_vproj4((m + 3) // 4)
                if it + 1 < 128:
                    emit_s(*divmod(it + 1, 32))
                emit_av(qi, m)
                if m == 31:
                    emit_norm(qi)
                    if qi == 1:
                        emit_outproj(0)
                    elif qi == 3:
                        emit_outproj(1)


_NC = None


def _get_program():
    global _NC
    if _NC is None:
        _NC = _build_program()
    return _NC


def kernel(x, qkv_w, proj_w, gamma, _trace=False):
    """Full inputs in, full output out. Shards across 8 NeuronCores internally."""
    nc = _get_program()
    B = x.shape[0]
    xf = np.ascontiguousarray(x.reshape(B, C, N).astype(np.float32))
    xf_bf = xf.astype(NPBF16)

    wqT = qkv_w[0:256].T.astype(NPBF16)
    wkT = qkv_w[256:512].T.astype(NPBF16)
    wvT = qkv_w[512:768].T.astype(NPBF16)
    pjT = proj_w.T.astype(NPBF16)
    # zero-padded proj tiles in the natural PSUM row layout of OsbR: for
    # g = 2*hg + j: rows 0-31 = head (4hg+2j) dims, 64-95 = head (4hg+2j+1)
    pjp = np.zeros((4, 128, 256), dtype=NPBF16)
    for g in range(4):
        hg, j = divmod(g, 2)
        h0 = 4 * hg + 2 * j
        pjp[g][0:32] = pjT[32 * h0:32 * (h0 + 1)]
        pjp[g][64:96] = pjT[32 * (h0 + 1):32 * (h0 + 2)]
    wall = np.ascontiguousarray(np.concatenate(
        [wqT[0:128], wqT[128:256], wkT[0:128], wkT[128:256],
         wvT[0:128], wvT[128:256], pjp[0], pjp[1], pjp[2], pjp[3]], axis=1))
    gam = np.full((128, 1), np.float32(gamma.reshape(-1)[0]), dtype=np.float32)

    in_maps = []
    for core in range(8):
        b, qi = divmod(core, 4)
        qs = slice(Q * qi, Q * (qi + 1))
        # rotate keys so this core's query block sits at columns 0-1023; key
        # order is irrelevant to attention (softmax + sum over keys).
        xrot = np.roll(xf_bf[b], -Q * qi, axis=1) if qi else xf_bf[b]
        in_maps.append({
            "xb": np.ascontiguousarray(xrot),
            "xq": np.ascontiguousarray(xf[b][:, qs]),
            "wall": wall,
            "gam": gam,
        })

    res = run_bass_kernel_spmd(nc, in_maps, core_ids=list(range(8)), trace=_trace)

    outf = np.empty((B, C, N), dtype=np.float32)
    for core in range(8):
        b, qi = divmod(core, 4)
        outf[b][:, Q * qi:Q * (qi + 1)] = res.results[core]["out"]
    result = outf.reshape(x.shape)
    if _trace:
        return result, res
    return result


# revision 4
# speedup vs baseline: 1.0387x; 1.0067x over previous
"""Multi-head self-attention 2d (B=2, C=256, H=W=64, 8 heads x 32 dim) on 8 TRN2 cores.

Sharding: batch (2-way) x query-rows-of-N=H*W (4-way) => 8 cores, no collectives.
v5: HAM-friendly quad boundaries --
  - the four (query-half, head-group) quads run as one continuous 128-iteration
    score/exp/AV pipeline; the next quad's score matmuls are emitted during the
    current quad's last iteration,
  - at each quad end the two PSUM accumulators are evacuated to SBUF with two
    quick copies (Op banks free in ~1.5us instead of ~4us), the reciprocal runs
    from SBUF with a -32-partition shift so every normalize multiply is
    partition-aligned, and the multiplies run on the otherwise-idle GpSimd
    engine; this kills the PE idle window at boundaries that was tripping the
    HAM clock throttle (PE at half clock for ~40us in v4),
  - normalized outputs stay in the natural PSUM row layout (denominator rows
    interleaved); the projection weights are zero-padded host-side at those
    rows so the output projection contracts over 512 rows with 4 matmuls,
  - xb is rotated per-core on the host so the query block is always columns
    0-1023 (key order is irrelevant to attention), keeping the program SPMD,
  - all weights ride in one [128, 2560] DMA; xb's first 1024 columns (feeding
    Q-proj and K chunks 0-1) transfer first,
  - K/V projections interleaved into the first quad, stealing score PSUM slots,
  - V stored as [V_h | ones32] so the AV matmul emits denominators pre-broadcast
    into 32 PSUM rows.
"""

import os
import sys

import numpy as np

for _p in ("/opt/trn_rl_repo", "/root/.axon_site/_ro/trn_rl_repo"):
    if os.path.isdir(_p) and _p not in sys.path:
        sys.path.insert(0, _p)

import ml_dtypes
import concourse.bacc as bacc
import concourse.bass as bass
import concourse.tile as tile
from concourse import mybir
from concourse.bass_utils import run_bass_kernel_spmd

BF16 = mybir.dt.bfloat16
F32 = mybir.dt.float32
I16 = mybir.dt.int16
NPBF16 = ml_dtypes.bfloat16

NH, D = 8, 32          # heads, head dim
C = 256                # channels
N = 4096               # H*W positions
Q = 1024               # query shard per core
SCALE = 1.0 / np.sqrt(D)

# Schraudolph bf16 exp on the vector engine: bf16_bits(exp(y)) ~= round(y*128/ln2
# + (127*128 - c)). Fold the attention scale into the multiplier. c tuned for
# min max-rel-error under round-to-nearest (~3.3%).
SCH_A = float(SCALE * 128.0 / np.log(2.0))
SCH_B = float(127.0 * 128.0 - 5.5)
# Fraction (num/32) of exp tiles on ScalarE: [quad0, steady]; quad 0 is
# ACT-heavy because DVE also drains the interleaved K/V projection copies.
ACT_UNITS = [21, 17]
TAIL_M = 31            # m >= TAIL_M goes all-ACT (DVE runs the normalize there)
TAIL_UNITS = 32
HEAD_M = 4             # first HEAD_M m-iters of quads 1-3 lean ACT (+HEAD_BUMP)
HEAD_BUMP = 7


def _build_program():
    nc = bacc.Bacc("TRN2", target_bir_lowering=False, debug=False)

    xb = nc.dram_tensor("xb", [C, N], BF16, kind="ExternalInput")
    xq = nc.dram_tensor("xq", [C, Q], F32, kind="ExternalInput")
    wall = nc.dram_tensor("wall", [128, 2560], BF16, kind="ExternalInput")
    gam = nc.dram_tensor("gam", [128, 1], F32, kind="ExternalInput")
    out = nc.dram_tensor("out", [C, Q], F32, kind="ExternalOutput")

    with tile.TileContext(nc) as tc:
        _emit(tc, xb, xq, wall, gam, out)
    nc.compile()
    return nc


def _emit(tc, xb, xq, wall, gam, out):
    from contextlib import ExitStack

    nc = tc.nc
    Exp = mybir.ActivationFunctionType.Exp

    with ExitStack() as ctx:
        per = ctx.enter_context(tc.tile_pool(name="persist", bufs=1))

        def ptile(name, shape, dtype):
            return per.tile(shape, dtype, name=name, tag=name)

        XB = [ptile(f"XB{i}", [128, N], BF16) for i in range(2)]
        XQ = [ptile(f"XQ{i}", [128, Q], F32) for i in range(2)]
        W = ptile("W", [128, 2560], BF16)   # wq01 wk01 wv01 pj[hg=0,j=0..1] pj[hg=1,...]
        WQ = [W[:, 256 * c:256 * (c + 1)] for c in range(2)]
        WK = [W[:, 512 + 256 * c:512 + 256 * (c + 1)] for c in range(2)]
        WV = [W[:, 1024 + 256 * c:1024 + 256 * (c + 1)] for c in range(2)]
        PJP = [W[:, 1536 + 256 * g:1536 + 256 * (g + 1)] for g in range(4)]  # (2hg+j)
        G = ptile("G", [128, 1], F32)
        Ksb = ptile("Ksb", [128, 2 * N], BF16)      # [dim-in-group, hg*4096 + key]
        Qsb = ptile("Qsb", [128, 2 * Q], BF16)      # [dim-in-group, hg*1024 + q]
        Vsb = ptile("Vsb", [128, 32 * 512], BF16)   # per m-chunk: 8 x [V(32)|ones(32)]
        # normalized attention outputs, natural PSUM row layout per (hg, j):
        # rows 0-31 head 2j, 32-63 junk, 64-95 head 2j+1, 96-127 junk
        OsbR = [ptile(f"OsbR{i}", [128, 2 * Q], BF16) for i in range(2)]

        # DMAs spread across engine DGE queues so the transfers run in
        # parallel (a single queue serializes at ~1.3us+transfer per DMA):
        # sync + scalar carry the two xb halves, vector carries the weights
        # (wq/wk first -- they gate Q/K-proj), gpsimd (idle, behind the
        # memsets) carries the late-needed xq/gam.
        nc.scalar.dma_start(W[:, 0:1024], wall[:, 0:1024])
        nc.sync.dma_start(XB[0][:, 0:1024], xb[0:128, 0:1024])
        nc.gpsimd.dma_start(XB[1][:, 0:1024], xb[128:256, 0:1024])
        nc.scalar.dma_start(W[:, 1024:2560], wall[:, 1024:2560])
        nc.sync.dma_start(XB[0][:, 1024:4096], xb[0:128, 1024:4096])
        nc.sync.dma_start(XB[1][:, 1024:4096], xb[128:256, 1024:4096])

        # one-time SBUF init on the idle GpSimd engine (behind its xb DMA):
        # ones blocks of Vsb and the junk rows of OsbR (so 0-padded proj rows
        # never hit NaN*0); then the late-needed xq/gam DMAs.
        v4 = Vsb.rearrange("p (mh w) -> p mh w", w=64)
        for m in range(32):
            nc.gpsimd.memset(v4[:, 8 * m:8 * (m + 1), 32:64], 1.0)
        for i in range(2):
            nc.gpsimd.memset(OsbR[i][32:64, :], 0.0)
            nc.gpsimd.memset(OsbR[i][96:128, :], 0.0)
        for i in range(2):
            r = slice(128 * i, 128 * (i + 1))
            nc.gpsimd.dma_start(XQ[i][:], xq[r, :])
        nc.gpsimd.dma_start(G[:], gam[:, :])

        exp_idx = [0]

        with ExitStack() as actx:
            sp = actx.enter_context(tc.tile_pool(name="sp", bufs=3, space="PSUM"))
            opl = actx.enter_context(tc.tile_pool(name="opl", bufs=1, space="PSUM"))
            pb = actx.enter_context(tc.tile_pool(name="pb", bufs=8))
            osb = actx.enter_context(tc.tile_pool(name="osb", bufs=2))
            rb = actx.enter_context(tc.tile_pool(name="rb", bufs=2))
            ob = actx.enter_context(tc.tile_pool(name="ob", bufs=2))

            def slot(name):
                return sp.tile([128, 1024], F32, name=name, tag="st2")

            def emit_qproj(p):
                qp = slot(f"qp{p}")
                for t2 in range(2):
                    ts_ = slice(512 * t2, 512 * (t2 + 1))
                    for c in range(2):
                        nc.tensor.matmul(qp[:, ts_], lhsT=WQ[c][:, 128 * p:128 * (p + 1)],
                                         rhs=XB[c][:, ts_], start=(c == 0), stop=(c == 1))
                nc.vector.tensor_copy(Qsb[:, 1024 * p:1024 * (p + 1)], qp[:])

            def emit_kproj(t):
                kp = slot(f"kp{t}")
                xs = slice(512 * t, 512 * (t + 1))
                for p in range(2):
                    ps_ = slice(512 * p, 512 * (p + 1))
                    for c in range(2):
                        nc.tensor.matmul(kp[:, ps_], lhsT=WK[c][:, 128 * p:128 * (p + 1)],
                                         rhs=XB[c][:, xs], start=(c == 0), stop=(c == 1))
                k3 = Ksb.rearrange("p (h w) -> p h w", w=N)
                nc.vector.tensor_copy(k3[:, :, 512 * t:512 * (t + 1)],
                                      kp[:].rearrange("p (h w) -> p h w", w=512))

            def emit_vproj4(mq):
                # chunks 4mq..4mq+3 packed into one slot at columns 256k
                vp = slot(f"vp{mq}")
                for k in range(4):
                    m = 4 * mq + k
                    ms = slice(128 * m, 128 * (m + 1))
                    vs = slice(256 * k, 256 * (k + 1))
                    nc.tensor.matmul(vp[:, vs], lhsT=XB[0][:, ms], rhs=WV[0][:],
                                     start=True, stop=False)
                    nc.tensor.matmul(vp[:, vs], lhsT=XB[1][:, ms], rhs=WV[1][:],
                                     start=False, stop=True)
                v3 = Vsb.rearrange("p (mh w) -> p mh w", w=64)
                nc.vector.tensor_copy(
                    v3[:, 32 * mq:32 * (mq + 1), 0:32],
                    vp[:].rearrange("p (kh d) -> p kh d", d=32))

            # flattened quad sequence: (qh, hg, units)
            quads = [(0, 0, ACT_UNITS[0]), (0, 1, ACT_UNITS[1]),
                     (1, 0, ACT_UNITS[1]), (1, 1, ACT_UNITS[1])]
            pts_by = {}
            Op_by = {}

            def emit_s(qi, m):
                qh, hg, units = quads[qi]
                if m >= TAIL_M:
                    u = TAIL_UNITS
                elif m < HEAD_M and qi > 0:
                    u = min(32, units + HEAD_BUMP)
                else:
                    u = units
                sts = [slot("st2s") for _ in range(2)]
                for g in range(2):
                    for j in range(2):
                        a = 2 * g + j
                        hh = slice(32 * a, 32 * (a + 1))
                        nc.tensor.matmul(
                            sts[g][:, 512 * j:512 * (j + 1)],
                            lhsT=Ksb[hh, N * hg + 128 * m:N * hg + 128 * (m + 1)],
                            rhs=Qsb[hh, Q * hg + 512 * qh:Q * hg + 512 * (qh + 1)],
                            start=True, stop=True,
                            tile_position=(32 * a, 0))
                pts = []
                for g in range(2):
                    pt2 = pb.tile([128, 1024], BF16, name="pt2", tag="pt2")
                    pts.append(pt2)
                    # accumulator Bresenham: even ACT/DVE interleave even as
                    # the ratio u changes across head/tail/quad transitions
                    exp_idx[0] += u
                    if exp_idx[0] >= 32:
                        exp_idx[0] -= 32
                        nc.scalar.activation(pt2[:], sts[g][:], Exp, scale=SCALE)
                    else:
                        nc.vector.tensor_scalar(
                            pt2.bitcast(I16)[:], sts[g][:], SCH_A, SCH_B,
                            mybir.AluOpType.mult, mybir.AluOpType.add)
                pts_by[(qi, m)] = pts

            def emit_av(qi, m):
                qh, hg, _ = quads[qi]
                if m == 0:
                    Op_by[qi] = [opl.tile([128, 512], F32, name=f"Op{j}", tag=f"Op{j}")
                                 for j in range(2)]
                Op = Op_by[qi]
                pts = pts_by.pop((qi, m))
                first, last = m == 0, m == 31
                for j in range(2):
                    for b in range(2):
                        a = 2 * j + b
                        H = 4 * hg + a
                        nc.tensor.matmul(
                            Op[j][64 * b:64 * (b + 1), :],
                            lhsT=Vsb[:, 512 * m + 64 * H:512 * m + 64 * (H + 1)],
                            rhs=pts[j][:, 512 * b:512 * (b + 1)],
                            start=first, stop=last,
                            tile_position=(0, 64 * b), skip_group_check=True)

            def emit_norm(qi):
                # rows 0-31: O head 2j; 32-63: denom head 2j (pre-broadcast by
                # the ones32 block of V); 64-95: O head 2j+1; 96-127: denom.
                qh, hg, _ = quads[qi]
                qs = slice(512 * qh, 512 * (qh + 1))
                Op = Op_by.pop(qi)
                if False:
                    # evacuate PSUM fast (Op banks free after 2 copies); then,
                    # off the Op critical path: shift the denominator rows down
                    # 32 with plain copies (custom DVE ops break on partition
                    # shifts; SBUF-SBUF tensor_tensor needs equal bases),
                    # aligned reciprocal, and fully-aligned GpSimd multiplies.
                    OS = osb.tile([128, 1024], F32, name="OS", tag="OS")
                    for j in range(2):
                        nc.vector.tensor_copy(OS[:, 512 * j:512 * (j + 1)], Op[j][:])
                    dn = rb.tile([128, 1024], F32, name="dn", tag="dn")
                    rj = rb.tile([128, 1024], F32, name="rj", tag="rj")
                    for b in range(2):
                        nc.vector.tensor_copy(dn[64 * b:64 * b + 32, :],
                                              OS[64 * b + 32:64 * b + 64, :])
                    # single base-0 op: reciprocal_approx_fast misreads at
                    # partition base 64 (rows 32-63 are junk-in, junk-out)
                    nc.vector.reciprocal_approx_fast(out=rj[0:96, :], in_=dn[0:96, :])
                    for j in range(2):
                        for b in range(2):
                            rr = slice(64 * b, 64 * b + 32)
                            nc.gpsimd.tensor_tensor(
                                OsbR[hg][rr, Q * j + 512 * qh:Q * j + 512 * (qh + 1)],
                                OS[rr, 512 * j:512 * j + 512],
                                rj[rr, 512 * j:512 * j + 512],
                                mybir.AluOpType.mult)
                else:
                    # tail quad: direct from PSUM on DVE (shortest latency)
                    for j in range(2):
                        rj = rb.tile([128, 1024], F32, name="rj", tag="rj")
                        nc.vector.reciprocal_approx_fast(out=rj[:, :512], in_=Op[j][:])
                        for b in range(2):
                            nc.vector.tensor_tensor(
                                OsbR[hg][64 * b:64 * b + 32,
                                         Q * j + 512 * qh:Q * j + 512 * (qh + 1)],
                                Op[j][64 * b:64 * b + 32, :],
                                rj[64 * b + 32:64 * b + 64, :512],
                                mybir.AluOpType.mult)

            def emit_outproj(qh):
                qs = slice(512 * qh, 512 * (qh + 1))
                for ct in range(2):
                    cs = slice(128 * ct, 128 * (ct + 1))
                    pp2 = slot(f"op{ct}")
                    for g in range(4):          # g = 2*hg + j
                        hg, j = divmod(g, 2)
                        nc.tensor.matmul(
                            pp2[:, :512], lhsT=PJP[g][:, cs],
                            rhs=OsbR[hg][:, Q * j + 512 * qh:Q * j + 512 * (qh + 1)],
                            start=(g == 0), stop=(g == 3))
                    obt = ob.tile([128, 512], F32, name="obt", tag="obt")
                    nc.vector.scalar_tensor_tensor(
                        obt[:], pp2[:, :512], G[:], XQ[ct][:, qs],
                        mybir.AluOpType.mult, mybir.AluOpType.add)
                    nc.sync.dma_start(out[cs, qs], obt[:])

            # projections needed up front: Q (both groups), K chunks 0-1, first
            # V quad-chunk; the rest interleaves into quad 0.
            emit_qproj(0)
            emit_qproj(1)
            emit_kproj(0)
            emit_s(0, 0)
            emit_kproj(1)
            emit_vproj4(0)
            for it in range(128):
                qi, m = divmod(it, 32)
                if qi == 0:
                    if m % 4 == 2 and m // 4 + 2 < 8:
                        emit_kproj(m // 4 + 2)
                    if m % 4 == 1 and m + 3 < 32:
                        emit_vproj4((m + 3) // 4)
                if it + 1 < 128:
                    emit_s(*divmod(it + 1, 32))
                emit_av(qi, m)
                if m == 31:
                    emit_norm(qi)
                    if qi == 1:
                        emit_outproj(0)
                    elif qi == 3:
                        emit_outproj(1)


_NC = None


def _get_program():
    global _NC
    if _NC is None:
        _NC = _build_program()
    return _NC


def kernel(x, qkv_w, proj_w, gamma, _trace=False):
    """Full inputs in, full output out. Shards across 8 NeuronCores internally."""
    nc = _get_program()
    B = x.shape[0]
    xf = np.ascontiguousarray(x.reshape(B, C, N).astype(np.float32))
    xf_bf = xf.astype(NPBF16)

    wqT = qkv_w[0:256].T.astype(NPBF16)
    wkT = qkv_w[256:512].T.astype(NPBF16)
    wvT = qkv_w[512:768].T.astype(NPBF16)
    pjT = proj_w.T.astype(NPBF16)
    # zero-padded proj tiles in the natural PSUM row layout of OsbR: for
    # g = 2*hg + j: rows 0-31 = head (4hg+2j) dims, 64-95 = head (4hg+2j+1)
    pjp = np.zeros((4, 128, 256), dtype=NPBF16)
    for g in range(4):
        hg, j = divmod(g, 2)
        h0 = 4 * hg + 2 * j
        pjp[g][0:32] = pjT[32 * h0:32 * (h0 + 1)]
        pjp[g][64:96] = pjT[32 * (h0 + 1):32 * (h0 + 2)]
    wall = np.ascontiguousarray(np.concatenate(
        [wqT[0:128], wqT[128:256], wkT[0:128], wkT[128:256],
         wvT[0:128], wvT[128:256], pjp[0], pjp[1], pjp[2], pjp[3]], axis=1))
    gam = np.full((128, 1), np.float32(gamma.reshape(-1)[0]), dtype=np.float32)

    in_maps = []
    for core in range(8):
        b, qi = divmod(core, 4)
        qs = slice(Q * qi, Q * (qi + 1))
        # rotate keys so this core's query block sits at columns 0-1023; key
        # order is irrelevant to attention (softmax + sum over keys).
        xrot = np.roll(xf_bf[b], -Q * qi, axis=1) if qi else xf_bf[b]
        in_maps.append({
            "xb": np.ascontiguousarray(xrot),
            "xq": np.ascontiguousarray(xf[b][:, qs]),
            "wall": wall,
            "gam": gam,
        })

    res = run_bass_kernel_spmd(nc, in_maps, core_ids=list(range(8)), trace=_trace)

    outf = np.empty((B, C, N), dtype=np.float32)
    for core in range(8):
        b, qi = divmod(core, 4)
        outf[b][:, Q * qi:Q * (qi + 1)] = res.results[core]["out"]
    result = outf.reshape(x.shape)
    if _trace:
        return result, res
    return result
